# revision 21
# baseline (speedup 1.0000x reference)
"""Multi-head QKV attention Trainium2 kernel (8-core SPMD).

Problem: B=2, N=M=2048, d_model=256, H=8 heads, d_head=32.
  q = queries @ Wq + bq ; k = keys @ Wk + bk ; v = values @ Wv + bv   (per-head slices)
  scores = q @ k^T  (per head), presence-masked softmax over keys, out = (attn @ v) @ Wo + bo

Sharding: B*H = 16 (batch, head) pairs -> 2 per core; core c handles batch b=c//4,
heads (2*(c%4), 2*(c%4)+1). Each core computes a partial output projection
sum over its 2 heads; the host sums the 4 per-batch core partials and adds bo.

Per-core dataflow (all layouts chosen so softmax reduces along PE/ACT-friendly axes):
  1. Load x=[2048,256] f32; PE-transpose 128x128 tiles -> xT (c-major) bf16.
  2. Projections on PE produce:
     - q_fold [128,512] per head: partition 32g+d = q^T[d, 512g+t] (n folded into groups)
     - k_rep  [128,2048] per head: k^T replicated on 4 partition groups (row-tile weights)
     - v      [m-part, d_v] with presence folded in (v*p), bias via K=1 ones-row matmul
  3. Scores^T per (head, m-chunk): 4-way row-tiled K=32 matmuls -> PSUM [128, 2048]
  4. ACT: exp(scale*s) PSUM->SBUF bf16 (one instr per chunk; mask handled by p-weighting)
  5. o'^T accumulation: col-tiled matmuls: rows 0-31/64-95 = unnormalized o^T per head,
     rows 32/96 = denominators (lhsT = presence column)
  6. Epilogue: divide via ACT Copy with per-partition scale = 1/d (d transposed via PE),
     u = o^T @ Wo per head, partial outputs DMA'd out.
"""

import numpy as np

B, N, M, C, H, D = 2, 2048, 2048, 256, 8, 32
NCORES = 8
SCALE = 1.0 / np.sqrt(32.0)

_CACHE = {}


def _build(main_reps=1, nch=16):
    """nch = number of 128-row key chunks after host-side compaction/padding.
    main_reps > 1 builds a timing-only variant (output wrong by ~reps factor)
    with the scores/exp/o' main loop repeated, for slope-based HW timing."""
    key = ("nc", main_reps, nch)
    if key in _CACHE:
        return _CACHE[key]

    import concourse.bass as bass
    import concourse.bacc as bacc
    import concourse.tile as tile
    from concourse import mybir

    f32 = mybir.dt.float32
    bf16 = mybir.dt.bfloat16
    AF = mybir.ActivationFunctionType
    OP = mybir.AluOpType

    nc = bacc.Bacc(
        "TRN2",
        target_bir_lowering=False,
        debug=False,
        enable_asserts=False,
        num_devices=NCORES,
    )

    # ---- DRAM I/O ----
    xq = nc.dram_tensor("xq", [N, C], f32, kind="ExternalInput").ap()
    xk = nc.dram_tensor("xk", [nch * 128, C], f32, kind="ExternalInput").ap()
    xv = nc.dram_tensor("xv", [nch * 128, C], f32, kind="ExternalInput").ap()
    wq_d = nc.dram_tensor("wq", [128, 2, 2 * D], f32, kind="ExternalInput").ap()
    wk_d = nc.dram_tensor("wk", [128, 2, 2 * D], f32, kind="ExternalInput").ap()
    wv_d = nc.dram_tensor("wv", [128, 2, 2 * D], f32, kind="ExternalInput").ap()
    bqf_d = nc.dram_tensor("bqf", [128, 2], f32, kind="ExternalInput").ap()
    bkf_d = nc.dram_tensor("bkf", [128, 2], f32, kind="ExternalInput").ap()
    bv_d = nc.dram_tensor("bv2", [1, 2 * D], f32, kind="ExternalInput").ap()
    wo_d = nc.dram_tensor("wo2", [128, C], f32, kind="ExternalInput").ap()
    pres_d = nc.dram_tensor("pres", [128, nch], f32, kind="ExternalInput").ap()
    id_d = nc.dram_tensor("ident", [128, 128], f32, kind="ExternalInput").ap()
    o0_d = nc.dram_tensor("o0", [N, C], f32, kind="ExternalOutput").ap()
    o1_d = nc.dram_tensor("o1", [N, C], f32, kind="ExternalOutput").ap()

    with tile.TileContext(nc) as tc:
        with (
            tc.tile_pool(name="sb", bufs=1) as sb,
            tc.tile_pool(name="xn", bufs=3) as xn,
            tc.tile_pool(name="expp", bufs=3) as expp,
            tc.tile_pool(name="ps", bufs=1, space="PSUM") as ps,
        ):
            # ---- constants / weights ----
            ident_sb = sb.tile([128, 128], f32, tag="ident")
            nc.sync.dma_start(out=ident_sb, in_=id_d)
            wq_bf = sb.tile([128, 2, 2 * D], bf16, tag="wqb")
            nc.gpsimd.dma_start(out=wq_bf, in_=wq_d)
            wk_bf = sb.tile([128, 2, 2 * D], bf16, tag="wkb")
            nc.gpsimd.dma_start(out=wk_bf, in_=wk_d)
            wv_bf = sb.tile([128, 2, 2 * D], bf16, tag="wvb")
            nc.gpsimd.dma_start(out=wv_bf, in_=wv_d)
            bv_bf = sb.tile([1, 2 * D], bf16, tag="bvb")
            nc.gpsimd.dma_start(out=bv_bf, in_=bv_d)
            bqf = sb.tile([128, 2], f32, tag="bqf")
            nc.sync.dma_start(out=bqf, in_=bqf_d)
            bkf = sb.tile([128, 2], f32, tag="bkf")
            nc.sync.dma_start(out=bkf, in_=bkf_d)
            wo_sb = sb.tile([128, C], f32, tag="wo")
            nc.sync.dma_start(out=wo_sb, in_=wo_d)
            pres_t = sb.tile([128, nch], f32, tag="pres")
            nc.sync.dma_start(out=pres_t, in_=pres_d)
            pT_bf = sb.tile([128, nch], bf16, tag="ptbf")
            nc.vector.tensor_copy(out=pT_bf, in_=pres_t)
            ones_bf = sb.tile([1, 128], bf16, tag="ones")
            nc.vector.memset(ones_bf, 1.0)

            # ---- load + transpose inputs to contraction-major ----
            xTs = []
            stage_i = 0
            for ti, xd in enumerate((xq, xk, xv)):
                tch = 16 if ti == 0 else nch
                x_nat = xn.tile([128, tch, C], f32, tag="xnat", name=f"xnat{ti}")
                xr = xd.rearrange("(t p) c -> p t c", p=128)
                for t0 in range(0, tch, 4):
                    t1 = min(t0 + 4, tch)
                    nc.sync.dma_start(out=x_nat[:, t0:t1, :], in_=xr[:, t0:t1, :])
                xT = sb.tile([128, 2, tch * 128], bf16, tag=f"xT{ti}")
                for ch in range(2):
                    stg = ps.tile(
                        [128, tch * 128],
                        f32,
                        tag=("quad" if stage_i % 2 == 0 else "oacc"),
                        name=f"stg{ti}_{ch}",
                    )
                    stage_i += 1
                    for t in range(tch):
                        nc.tensor.transpose(
                            out=stg[:, 128 * t : 128 * t + 128],
                            in_=x_nat[:, t, 128 * ch : 128 * ch + 128],
                            identity=ident_sb,
                        )
                    nc.vector.tensor_copy(out=xT[:, ch, :], in_=stg)
                xTs.append(xT)
            qT, kT, vT = xTs

            # ---- q_fold projection (both heads in one [128,1024] psum tile) ----
            q_fold_ps = ps.tile([128, 1024], f32, tag="quad", name="q_fold_ps")
            for h in range(2):
                for g in range(4):
                    for ch in range(2):
                        nc.tensor.matmul(
                            q_fold_ps[32 * g : 32 * g + 32, 512 * h : 512 * h + 512],
                            lhsT=wq_bf[:, ch, 32 * h : 32 * h + 32],
                            rhs=qT[:, ch, 512 * g : 512 * g + 512],
                            start=(ch == 0),
                            stop=(ch == 1),
                            tile_position=(0, 32 * g),
                        )
            q_fold_sb = sb.tile([128, 1024], bf16, tag="qfold")
            for h in range(2):
                nc.vector.tensor_scalar(
                    out=q_fold_sb[:, 512 * h : 512 * h + 512],
                    in0=q_fold_ps[:, 512 * h : 512 * h + 512],
                    scalar1=bqf[:, h : h + 1],
                    scalar2=None,
                    op0=OP.add,
                )

            # ---- k_rep projection (k^T replicated across 4 partition groups) ----
            k_rep_sb = []
            for h in range(2):
                k_ps = ps.tile(
                    [128, nch * 128],
                    f32,
                    tag=("oacc" if h == 0 else "quad"),
                    name=f"k_ps{h}",
                )
                for g in range(4):
                    for j in range(nch // 4):
                        for ch in range(2):
                            nc.tensor.matmul(
                                k_ps[32 * g : 32 * g + 32, 512 * j : 512 * j + 512],
                                lhsT=wk_bf[:, ch, 32 * h : 32 * h + 32],
                                rhs=kT[:, ch, 512 * j : 512 * j + 512],
                                start=(ch == 0),
                                stop=(ch == 1),
                                tile_position=(0, 32 * g),
                            )
                krs = sb.tile([128, nch * 128], bf16, tag=f"krep{h}", name=f"krep{h}")
                nc.vector.tensor_scalar(
                    out=krs,
                    in0=k_ps,
                    scalar1=bkf[:, h : h + 1],
                    scalar2=None,
                    op0=OP.add,
                )
                k_rep_sb.append(krs)

            # ---- v projection ([m-part, 64] per m-chunk; bias via ones-row) ----
            v_ps = ps.tile([128, 64 * nch], f32, tag="oacc", name="v_ps")
            for mc in range(nch):
                for ch in range(2):
                    nc.tensor.matmul(
                        v_ps[:, 64 * mc : 64 * mc + 64],
                        lhsT=vT[:, ch, 128 * mc : 128 * mc + 128],
                        rhs=wv_bf[:, ch, :],
                        start=(ch == 0),
                        stop=False,
                    )
                nc.tensor.matmul(
                    v_ps[:, 64 * mc : 64 * mc + 64],
                    lhsT=ones_bf[0:1, 0:128],
                    rhs=bv_bf[0:1, :],
                    start=False,
                    stop=True,
                )
            # v_aug[:, mc, h, 0:32] = v_h * presence; col 32 = presence
            # (appended column makes the o' matmul also produce the softmax
            # denominator at output row 64h+32)
            v_aug = sb.tile([128, nch, 2, D + 1], bf16, tag="vaug")
            for mc in range(nch):
                for h in range(2):
                    nc.vector.tensor_scalar(
                        out=v_aug[:, mc, h, 0:D],
                        in0=v_ps[:, 64 * mc + 32 * h : 64 * mc + 32 * h + 32],
                        scalar1=pres_t[:, mc : mc + 1],
                        scalar2=None,
                        op0=OP.mult,
                    )
            for h in range(2):
                nc.vector.tensor_copy(
                    out=v_aug[:, :, h, D : D + 1].rearrange("p a b -> p (a b)"),
                    in_=pT_bf,
                )

            # ---- main loop: scores^T -> exp -> o' accumulation ----
            # Software-pipelined: the o' matmuls for iteration i-1 are emitted
            # after iteration i's score pack + exp, so ACT's exp overlaps PE's
            # o' work instead of serializing scores -> exp -> o' per iteration.
            oacc = ps.tile([128, 2048], f32, tag="oacc", name="oacc")
            iters = [
                (r, m, h)
                for r in range(main_reps)
                for m in range(nch)
                for h in range(2)
            ]

            def emit_oprime(ex, mc, h, first, last):
                # first/last refer to this head's accumulation series
                for j in range(4):
                    nc.tensor.matmul(
                        oacc[64 * h : 64 * h + 33, 512 * j : 512 * j + 512],
                        lhsT=v_aug[:, mc, h, :],
                        rhs=ex[:, 512 * j : 512 * j + 512],
                        start=first,
                        stop=last,
                        skip_group_check=True,
                        tile_position=(0, 64 * h),
                    )

            def emit_exp(sc, it):
                # two half-tile exps: subtile WAR lets the next score pack's
                # first row-groups start while ACT still exps the second half
                ex = expp.tile([128, 2048], bf16, tag="exp", name=f"ex{it}")
                for half in range(2):
                    nc.scalar.activation(
                        out=ex[:, 1024 * half : 1024 * half + 1024],
                        in_=sc[:, 1024 * half : 1024 * half + 1024],
                        func=AF.Exp,
                        bias=0.0,
                        scale=float(SCALE),
                    )
                return ex

            prev = None
            for it, (rep, mc, h) in enumerate(iters):
                sc = ps.tile([128, 2048], f32, tag="quad", name=f"sc{it}")
                for g in range(4):
                    nc.tensor.matmul(
                        sc[:, 512 * g : 512 * g + 512],
                        lhsT=k_rep_sb[h][
                            32 * g : 32 * g + 32, 128 * mc : 128 * mc + 128
                        ],
                        rhs=q_fold_sb[32 * g : 32 * g + 32, 512 * h : 512 * h + 512],
                        start=True,
                        stop=True,
                        tile_position=(32 * g, 0),
                    )
                ex = emit_exp(sc, it)
                if prev is not None:
                    emit_oprime(*prev)
                prev = (
                    ex,
                    mc,
                    h,
                    rep == 0 and mc == 0,
                    rep == main_reps - 1 and mc == nch - 1,
                )
            emit_oprime(*prev)

            # ---- epilogue ----
            o_sb = sb.tile([128, 2048], f32, tag="osb")
            nc.vector.tensor_copy(out=o_sb, in_=oacc)
            d2 = sb.tile([2, 2048], f32, tag="d2")
            nc.sync.dma_start(out=d2[0:1, :], in_=o_sb[32:33, :])
            nc.sync.dma_start(out=d2[1:2, :], in_=o_sb[96:97, :])
            dT = ps.tile([128, 32], f32, tag="oacc", name="dT")
            for j in range(16):
                nc.tensor.transpose(
                    out=dT[:, 2 * j : 2 * j + 2],
                    in_=d2[0:2, 128 * j : 128 * j + 128],
                    identity=ident_sb[0:2, 0:2],
                )
            r_sb = sb.tile([128, 32], f32, tag="rsb")
            nc.vector.reciprocal(out=r_sb, in_=dT)

            out_sb = [
                xn.tile([128, 16, C], f32, tag="xnat", name=f"out{h}") for h in range(2)
            ]
            o_dr = [
                o0_d.rearrange("(t p) c -> p t c", p=128),
                o1_d.rearrange("(t p) c -> p t c", p=128),
            ]
            for j in range(16):
                u_ps = ps.tile([128, 1024], f32, tag="quad", name=f"u{j}")
                for h in range(2):
                    nc.tensor.matmul(
                        u_ps[:, 512 * h : 512 * h + 256],
                        lhsT=o_sb[64 * h : 64 * h + 32, 128 * j : 128 * j + 128],
                        rhs=wo_sb[64 * h : 64 * h + 32, :],
                        start=True,
                        stop=True,
                        tile_position=(64 * h, 0),
                    )
                for h in range(2):
                    nc.scalar.activation(
                        out=out_sb[h][:, j, :],
                        in_=u_ps[:, 512 * h : 512 * h + 256],
                        func=AF.Copy,
                        bias=0.0,
                        scale=r_sb[:, 2 * j + h : 2 * j + h + 1],
                    )
                    nc.sync.dma_start(out=o_dr[h][:, j, :], in_=out_sb[h][:, j, :])

    nc.compile()
    _CACHE[key] = nc
    return nc


def _plan_compaction(presence):
    """Host-side key compaction: drop fully-masked key rows, pad to a
    512-row multiple shared by both batches (SPMD cores share one program)."""
    idxs = [np.where(np.asarray(presence[b]) > 0)[0] for b in range(B)]
    mc = max(len(ix) for ix in idxs)
    nch = min(16, max(4, 4 * ((mc + 511) // 512)))
    return idxs, nch


def _core_inputs(inputs, c, idxs, nch):
    b, p = c // 4, c % 4
    h0 = 2 * p
    hsl = slice(h0 * D, (h0 + 2) * D)
    f = np.float32
    Wq, Wk, Wv, Wo = (np.asarray(inputs[k], f) for k in ("Wq", "Wk", "Wv", "Wo"))
    bq, bk = np.asarray(inputs["bq"], f), np.asarray(inputs["bk"], f)
    bv = np.asarray(inputs["bv"], f)
    idx = idxs[b]
    mrows = nch * 128
    keys_c = np.zeros((mrows, C), f)
    keys_c[: len(idx)] = np.asarray(inputs["keys"], f)[b][idx]
    values_c = np.zeros((mrows, C), f)
    values_c[: len(idx)] = np.asarray(inputs["values"], f)[b][idx]
    pres = np.zeros(mrows, f)
    pres[: len(idx)] = 1.0

    def chunkw(W):
        # [256, 64] -> [128, 2, 64] with element (p_, ch, d) = W[ch*128+p_, d]
        return np.ascontiguousarray(W[:, hsl].reshape(2, 128, 2 * D).transpose(1, 0, 2))

    def fold_bias(bias):
        # per-head [128, 2]: partition 32g+d (g=0..3) = bias[h*32+d]
        out = np.zeros((128, 2), f)
        for h in range(2):
            out[:, h] = np.tile(bias[hsl][h * D : (h + 1) * D], 4)
        return out

    wo2 = np.zeros((128, C), f)
    wo2[0:32] = Wo[hsl][0:D]
    wo2[64:96] = Wo[hsl][D : 2 * D]
    return {
        "xq": np.ascontiguousarray(np.asarray(inputs["queries"], f)[b]),
        "xk": keys_c,
        "xv": values_c,
        "wq": chunkw(Wq),
        "wk": chunkw(Wk),
        "wv": chunkw(Wv),
        "bqf": fold_bias(bq),
        "bkf": fold_bias(bk),
        "bv2": np.ascontiguousarray(bv[hsl][None, :]),
        "wo2": wo2,
        "pres": np.ascontiguousarray(pres.reshape(nch, 128).T),
        "ident": np.eye(128, dtype=f),
    }


def make_in_maps(inputs):
    idxs, nch = _plan_compaction(np.asarray(inputs["presence"]))
    return [_core_inputs(inputs, c, idxs, nch) for c in range(NCORES)], nch


def kernel(**inputs):
    from concourse.bass_utils import run_bass_kernel_spmd

    in_maps, nch = make_in_maps(inputs)
    nc = _build(nch=nch)
    res = run_bass_kernel_spmd(nc, in_maps, core_ids=list(range(NCORES)))
    bo = np.asarray(inputs["bo"], np.float32)
    out = np.zeros((B, N, C), np.float32)
    for c in range(NCORES):
        out[c // 4] += res.results[c]["o0"] + res.results[c]["o1"]
    out += bo[None, None, :]
    return out
